# revision 19
# baseline (speedup 1.0000x reference)
"""DGCNN segmentation kernel for 8 Trainium2 NeuronCores.

Sharding: core c owns batch b = c // 4 and query chunk q = c % 4 (1024
queries).  Each layer: distances + exact top-20 selection (DVE max8 rounds)
+ indirect-DMA gather of projected neighbor features + max-over-k reduce.
Features are exchanged between the 4 cores of a batch with an AllGather
after layers 1-3; the global-max-pool vector is combined with an
AllReduce(max).

EdgeConv restructuring (host folds s into weights):
  y[n,k,:] = s * (W @ [nbr - xc; xc]) + b = An @ nbr + Qn @ xc + b
  with An = s*W[:, :C],  Qn = s*(W[:, C:] - W[:, :C]).
  max_k lrelu(y) = lrelu(max_k (An @ nbr) + Qn @ xc + b)   (monotonic).
"""
import sys

sys.path.insert(0, "/opt/trn_rl_repo")

import numpy as np

import concourse.bacc as bacc
import concourse.bass as bass
import concourse.mybir as mybir
import concourse.tile as tile
from concourse import masks

F32 = mybir.dt.float32
U32 = mybir.dt.uint32
AF = mybir.ActivationFunctionType
ALU = mybir.AluOpType

SEL_MODE = "sliced"   # "full" = 3x(max+mi+mr) full-width; "sliced" = 32x128 hierarchy
MARK = -3.0e38

N, K, B = 4096, 20, 2
NCHUNK = 4          # query chunks per batch
QN = N // NCHUNK    # queries per core
P = 128             # queries per tile
QT = QN // P        # query tiles per core
LAYERS = [(3, 64), (64, 64), (64, 128), (128, 256)]  # (C, O)
EMB = 1024
GROUPS = [[0, 1, 2, 3], [4, 5, 6, 7]]


def build_program():
    nc = bacc.Bacc("TRN2", target_bir_lowering=False, debug=False, num_devices=8)

    ins = {}

    def inp(name, shape):
        ins[name] = nc.dram_tensor(name, list(shape), F32, kind="ExternalInput")
        return ins[name]

    xfT = inp("xfT", [3, N])
    xqT = inp("xqT", [3, QN])
    for li, (C, O) in enumerate(LAYERS):
        inp(f"AnT{li}", [C, O])
        inp(f"QnT{li}", [C, O])
        inp(f"brep{li}", [P, O])
    WgT = inp("WgT", [512, EMB])
    W1aT = inp("W1aT", [512, 512])
    W1bT = inp("W1bT", [EMB, 512])
    W2T = inp("W2T", [512, 256])
    W3T = inp("W3T", [256, 16])      # 13 padded to 16 cols
    sgr = inp("sgr", [P, 8])
    bgr = inp("bgr", [P, 8])
    s1r = inp("s1r", [P, 4])
    b1r = inp("b1r", [P, 4])
    s2r = inp("s2r", [P, 2])
    b2r = inp("b2r", [P, 2])
    b3r = inp("b3r", [16, 1])

    outd = nc.dram_tensor("out", [QN, 16], F32, kind="ExternalOutput")
    dbg_s = nc.dram_tensor("dbg_s", [P, 512], F32, kind="ExternalOutput")
    dbg_nidx = nc.dram_tensor("dbg_nidx", [P, 20], U32, kind="ExternalOutput")
    dbg_zg = nc.dram_tensor("dbg_zg", [P, K * 64], F32, kind="ExternalOutput")
    dbg_x1 = nc.dram_tensor("dbg_x1", [64, QN], F32, kind="ExternalOutput")
    dbg_xt2 = nc.dram_tensor("dbg_xt2", [64, N], F32, kind="ExternalOutput")

    zdram = [nc.dram_tensor(f"zdram{li}", [N, O], F32) for li, (C, O) in enumerate(LAYERS)]
    ccin = [nc.dram_tensor(f"ccin{li}", [O, QN], F32)
            for li, (C, O) in enumerate(LAYERS[:3])]
    ccout = [nc.dram_tensor(f"ccout{li}", [NCHUNK, O, QN], F32)
             for li, (C, O) in enumerate(LAYERS[:3])]
    gin = nc.dram_tensor("gin", [8, P], F32)
    gout = nc.dram_tensor("gout", [8, P], F32)

    with tile.TileContext(nc) as tc:
        with (
            tc.tile_pool(name="perst", bufs=1) as perst,
            tc.tile_pool(name="wpool", bufs=1) as wpool,
        ):
            ident = perst.tile([128, 128], F32)
            masks.make_identity(nc, ident[:])
            onesP = perst.tile([128, 1], F32)
            nc.vector.memset(onesP[:], 1.0)
            ones1 = perst.tile([1, P], F32)
            nc.vector.memset(ones1[:], 1.0)

            # feature planes (transposed, this core's queries)
            x1T = perst.tile([64, QN], F32)
            x2T = perst.tile([64, QN], F32)
            plane2 = perst.tile([128, QN], F32)    # x3T
            plane3a = perst.tile([128, QN], F32)   # x4T[0:128]
            plane3b = perst.tile([128, QN], F32)   # x4T[128:256]
            xq_sb = perst.tile([3, QN], F32)
            nc.sync.dma_start(xq_sb[:], xqT[:])

            xT = perst.tile([128, N], F32)         # keys (C rows used)
            nxx = perst.tile([1, N], F32)          # -|x|^2 row

            if SEL_MODE == "sliced":
                sbase_i = perst.tile([128, 256], mybir.dt.int32)
                nc.gpsimd.iota(sbase_i[:], pattern=[[128, 32], [0, 8]],
                               base=0, channel_multiplier=0)
                sbase = perst.tile([128, 256], F32)
                nc.vector.tensor_copy(sbase[:], sbase_i[:])
                negbig = perst.tile([128, 256], F32)
                nc.vector.memset(negbig[:], MARK)

            my_src = {0: xq_sb[0:3, :], 1: x1T[:, :],
                      2: x2T[:, :], 3: plane2[0:128, :]}
            out_planes = {0: [x1T[:, :]], 1: [x2T[:, :]],
                          2: [plane2[0:128, :]], 3: [plane3a[0:128, :], plane3b[0:128, :]]}

            AnT_sb, QnT_sb, brep_sb = {}, {}, {}
            for li, (C, O) in enumerate(LAYERS):
                AnT_sb[li] = wpool.tile([C, O], F32, tag=f"AnT{li}", name=f"AnT{li}_sb")
                nc.sync.dma_start(AnT_sb[li][:], ins[f"AnT{li}"][:])
                QnT_sb[li] = wpool.tile([C, O], F32, tag=f"QnT{li}", name=f"QnT{li}_sb")
                nc.sync.dma_start(QnT_sb[li][:], ins[f"QnT{li}"][:])
                brep_sb[li] = wpool.tile([P, O], F32, tag=f"brep{li}", name=f"brep{li}_sb")
                nc.sync.dma_start(brep_sb[li][:], ins[f"brep{li}"][:])

            # ---------------- layers ----------------
            with (
                tc.tile_pool(name="lay", bufs=1) as lay,
                tc.tile_pool(name="Spool", bufs=2) as Spool,
                tc.tile_pool(name="zgpool", bufs=2) as zgpool,
                tc.tile_pool(name="small", bufs=2) as small,
                tc.tile_pool(name="ps1", bufs=1, space="PSUM") as ps1,
                tc.tile_pool(name="ps2", bufs=2, space="PSUM") as ps2,
            ):
                for li, (C, O) in enumerate(LAYERS):
                    # --- keys into xT ---
                    if li == 0:
                        nc.sync.dma_start(xT[0:3, :], xfT[:])
                    else:
                        cco = ccout[li - 1]
                        nc.sync.dma_start(
                            xT[0:C, :].rearrange("c (r j) -> c r j", r=NCHUNK),
                            cco[:, :, :].rearrange("r c j -> c r j"),
                        )
                    # --- -xx row ---
                    sq = lay.tile([128, N], F32, tag="sq")
                    nc.scalar.activation(sq[0:C, :], xT[0:C, :], AF.Square)
                    for ch in range(8):
                        sl = slice(ch * 512, (ch + 1) * 512)
                        pxx = ps1.tile([1, 512], F32, tag="pxx")
                        nc.tensor.matmul(pxx[:], onesP[0:C, :], sq[0:C, sl])
                        nc.scalar.activation(nxx[:, sl], pxx[:], AF.Copy, scale=-1.0)
                    # --- Z rows -> DRAM ---
                    for j in range(N // 128):
                        jsl = slice(j * 128, (j + 1) * 128)
                        pz = ps2.tile([128, O], F32, tag="pz")
                        nc.tensor.matmul(pz[:], xT[0:C, jsl], AnT_sb[li][:])
                        zrow = small.tile([128, O], F32, tag="zrow")
                        nc.scalar.activation(zrow[:], pz[:], AF.Copy)
                        nc.sync.dma_start(zdram[li][jsl, :], zrow[:])
                    # --- query tiles ---
                    for t in range(QT):
                        tsl = slice(t * P, (t + 1) * P)
                        src = my_src[li]
                        lhs2 = small.tile([C, P], F32, tag="lhs2")
                        nc.scalar.activation(lhs2[:], src[:, tsl], AF.Copy, scale=2.0)

                        S = Spool.tile([P, N], F32, tag="S")
                        for ch in range(8):
                            sl = slice(ch * 512, (ch + 1) * 512)
                            pd = ps2.tile([P, 512], F32, tag="pd")
                            nc.tensor.matmul(pd[:], lhs2[:], xT[0:C, sl],
                                             start=True, stop=False)
                            nc.tensor.matmul(pd[:], ones1[:], nxx[:, sl],
                                             start=False, stop=True)
                            nc.scalar.activation(S[:, sl], pd[:], AF.Copy)

                        NIDX = small.tile([P, 24], U32, tag="NIDX")
                        if SEL_MODE == "full":
                            V = small.tile([P, 8], F32, tag="V")
                            scr = small.tile([P, 8], U32, tag="scr")
                            for r in range(3):
                                nc.vector.max(V[:], S[:])
                                if r < 2:
                                    nc.vector.max_index(NIDX[:, r * 8:(r + 1) * 8], V[:], S[:])
                                    nc.vector.match_replace(S[:], V[:], S[:], MARK)
                                else:
                                    nc.vector.max_index(scr[:], V[:], S[:])
                            nc.vector.tensor_copy(NIDX[:, 16:20], scr[:, 0:4])
                        else:
                            # per-slice top-8 (exact: P(>8 of top-20 in one
                            # 128-slice) ~ 1.5e-7/row)
                            V16 = small.tile([P, 256], F32, tag="V16")
                            I16 = small.tile([P, 256], U32, tag="I16")
                            for s in range(32):
                                ssl = slice(s * 128, (s + 1) * 128)
                                osl = slice(s * 8, (s + 1) * 8)
                                nc.vector.max(V16[:, osl], S[:, ssl])
                            for s in range(32):
                                ssl = slice(s * 128, (s + 1) * 128)
                                osl = slice(s * 8, (s + 1) * 8)
                                nc.vector.max_index(I16[:, osl], V16[:, osl], S[:, ssl])
                            GIDX = small.tile([P, 256], F32, tag="GIDX")
                            nc.vector.tensor_copy(GIDX[:], I16[:])
                            nc.vector.tensor_tensor(GIDX[:], GIDX[:], sbase[:], op=ALU.add)
                            # merge: mark the top-20 values in V16
                            Vm = small.tile([P, 8], F32, tag="Vm")
                            nc.vector.max(Vm[:], V16[:])
                            nc.vector.match_replace(V16[:], Vm[:], V16[:], MARK)
                            nc.vector.max(Vm[:], V16[:])
                            nc.vector.match_replace(V16[:], Vm[:], V16[:], MARK)
                            nc.vector.max(Vm[:], V16[:])
                            rep8 = small.tile([P, 8], F32, tag="rep8")
                            nc.vector.memset(rep8[:], 3.0e38)
                            nc.vector.tensor_copy(rep8[:, 0:4], Vm[:, 0:4])
                            nc.vector.match_replace(V16[:], rep8[:], V16[:], MARK)
                            # indices of marked entries
                            maskt = small.tile([P, 256], mybir.dt.uint8, tag="maskt")
                            nc.vector.tensor_scalar(maskt[:], V16[:], MARK, None,
                                                    op0=ALU.is_equal)
                            IDXm = small.tile([P, 256], F32, tag="IDXm")
                            nc.vector.tensor_copy(IDXm[:], negbig[:])
                            nc.vector.copy_predicated(IDXm[:], maskt[:], GIDX[:])
                            NIDXf = small.tile([P, 24], F32, tag="NIDXf")
                            nc.vector.max(NIDXf[:, 0:8], IDXm[:])
                            nc.vector.match_replace(IDXm[:], NIDXf[:, 0:8], IDXm[:], MARK)
                            nc.vector.max(NIDXf[:, 8:16], IDXm[:])
                            nc.vector.match_replace(IDXm[:], NIDXf[:, 8:16], IDXm[:], MARK)
                            nc.vector.max(NIDXf[:, 16:24], IDXm[:])
                            nc.vector.tensor_copy(NIDX[:, 0:20], NIDXf[:, 0:20])

                        if li == 0 and t == 0:
                            nc.sync.dma_start(dbg_s[:, :], S[:, 0:512])
                            nc.sync.dma_start(dbg_nidx[:, :], NIDX[:, 0:20])
                        zg = zgpool.tile([P, K * O], F32, tag="zg")
                        # One offset per partition per instruction: the only
                        # layout the HW dynamic-AP path handles correctly.
                        for kk in range(K):
                            nc.gpsimd.indirect_dma_start(
                                out=zg[:, kk * O:(kk + 1) * O],
                                out_offset=None,
                                in_=zdram[li][:, :],
                                in_offset=bass.IndirectOffsetOnAxis(
                                    ap=NIDX[:, kk:kk + 1], axis=0),
                            )

                        if li == 0 and t == 0:
                            nc.sync.dma_start(dbg_zg[:, :], zg[:, :])
                        pq = ps1.tile([P, O], F32, tag="pq")
                        nc.tensor.matmul(pq[:], src[:, tsl], QnT_sb[li][:])
                        M = small.tile([P, O], F32, tag="M")
                        nc.vector.tensor_reduce(
                            M[:], zg[:].rearrange("p (k o) -> p o k", k=K),
                            axis=mybir.AxisListType.X, op=ALU.max)
                        y = small.tile([P, O], F32, tag="y")
                        nc.vector.tensor_tensor(y[:], M[:], pq[:], op=ALU.add)
                        nc.vector.tensor_tensor(y[:], y[:], brep_sb[li][:], op=ALU.add)
                        u = small.tile([P, O], F32, tag="u")
                        nc.scalar.activation(u[:], y[:], AF.Copy, scale=0.2)
                        xo = small.tile([P, O], F32, tag="xo")
                        nc.vector.tensor_tensor(xo[:], y[:], u[:], op=ALU.max)

                        for pl, dest in enumerate(out_planes[li]):
                            pt = ps1.tile([min(O, 128), P], F32, tag="pt")
                            nc.tensor.transpose(
                                pt[:], xo[:, pl * 128:pl * 128 + min(O, 128)], ident[:])
                            nc.scalar.activation(dest[:, tsl], pt[:], AF.Copy)

                    if li == 0:
                        nc.sync.dma_start(dbg_x1[:, :], x1T[:, :])
                    if li == 1:
                        nc.sync.dma_start(dbg_xt2[:, :], xT[0:64, :])
                    if li < 3:
                        nc.sync.dma_start(ccin[li][:, :], out_planes[li][0])
                        nc.gpsimd.collective_compute(
                            "AllGather", ALU.bypass, replica_groups=GROUPS,
                            ins=[ccin[li][:, :]], outs=[ccout[li][:, :, :]])

            # ---------------- head ----------------
            # K-blocks of xcat: (plane AP, weight-row offset, size)
            xcb = [(x1T[:, :], 0, 64), (x2T[:, :], 64, 64), (plane2[:, :], 128, 128),
                   (plane3a[:, :], 256, 128), (plane3b[:, :], 384, 128)]
            with (
                tc.tile_pool(name="hw", bufs=1) as hw,
                tc.tile_pool(name="hact", bufs=1) as hact,
                tc.tile_pool(name="htmp", bufs=2) as htmp,
                tc.tile_pool(name="hps", bufs=2, space="PSUM") as hps,
                tc.tile_pool(name="hps1", bufs=1, space="PSUM") as hps1,
            ):
                WgT_sb = [hw.tile([sz, EMB], F32, tag=f"wg{kb}", name=f"wg{kb}_sb")
                          for kb, (_, off, sz) in enumerate(xcb)]
                for kb, (_, off, sz) in enumerate(xcb):
                    nc.sync.dma_start(WgT_sb[kb][:], WgT[off:off + sz, :])
                scl = {}
                for nm, t_ in (("sgr", sgr), ("bgr", bgr), ("s1r", s1r), ("b1r", b1r),
                               ("s2r", s2r), ("b2r", b2r)):
                    scl[nm] = hw.tile(list(t_.shape), F32, tag=nm, name=nm + "_sb")
                    nc.sync.dma_start(scl[nm][:], t_[:])
                b3_sb = hw.tile([16, 1], F32, tag="b3")
                nc.sync.dma_start(b3_sb[:], b3r[:])

                gmax = hact.tile([P, 8], F32)
                for i in range(8):
                    isl = slice(i * 128, (i + 1) * 128)
                    gT = htmp.tile([128, QN], F32, tag="gT")
                    for cc in range(QN // 512):
                        csl = slice(cc * 512, (cc + 1) * 512)
                        pg = hps.tile([128, 512], F32, tag="mm")
                        for kb, (pl_, off, sz) in enumerate(xcb):
                            nc.tensor.matmul(pg[:], WgT_sb[kb][:, isl], pl_[:, csl],
                                             start=(kb == 0), stop=(kb == len(xcb) - 1))
                        # t = sg*mm + bg ; lrelu
                        tt_ = htmp.tile([128, 512], F32, tag="tt")
                        nc.scalar.activation(tt_[:], pg[:], AF.Identity,
                                             scale=scl["sgr"][:, i:i + 1],
                                             bias=scl["bgr"][:, i:i + 1])
                        uu = htmp.tile([128, 512], F32, tag="uu")
                        nc.scalar.activation(uu[:], tt_[:], AF.Copy, scale=0.2)
                        nc.vector.tensor_tensor(gT[:, csl], tt_[:], uu[:], op=ALU.max)
                    nc.vector.tensor_reduce(gmax[:, i:i + 1], gT[:],
                                            axis=mybir.AxisListType.X, op=ALU.max)
                # AllReduce max over the 4 cores of the batch
                pgt = hps1.tile([8, P], F32, tag="tr")
                nc.tensor.transpose(pgt[:], gmax[:], ident[:])
                gt8 = hact.tile([8, P], F32)
                nc.scalar.activation(gt8[:], pgt[:], AF.Copy)
                nc.sync.dma_start(gin[:, :], gt8[:])
                nc.gpsimd.collective_compute(
                    "AllReduce", ALU.max, replica_groups=GROUPS,
                    ins=[gin[:, :]], outs=[gout[:, :]])
                gld = hact.tile([8, P], F32)
                nc.sync.dma_start(gld[:], gout[:, :])
                pgb = hps1.tile([P, 8], F32, tag="tr")
                nc.tensor.transpose(pgb[:], gld[:], ident[0:8, 0:8])
                gsb = hact.tile([P, 8], F32)
                nc.scalar.activation(gsb[:], pgb[:], AF.Copy)

                W1bT_sb = [hw.tile([128, 512], F32, tag=f"w1b{i}", name=f"w1b{i}_sb") for i in range(8)]
                for i in range(8):
                    nc.sync.dma_start(W1bT_sb[i][:], W1bT[i * 128:(i + 1) * 128, :])
                # bias' = s1*(W1b@g) + b1 per out-plane j
                biasp = hact.tile([P, 4], F32)
                for j in range(4):
                    jsl = slice(j * 128, (j + 1) * 128)
                    pb = hps1.tile([P, 1], F32, tag="pb")
                    for i in range(8):
                        nc.tensor.matmul(pb[:], W1bT_sb[i][:, jsl], gsb[:, i:i + 1],
                                         start=(i == 0), stop=(i == 7))
                    nc.vector.scalar_tensor_tensor(
                        out=biasp[:, j:j + 1], in0=pb[:], scalar=scl["s1r"][:, j:j + 1],
                        in1=scl["b1r"][:, j:j + 1], op0=ALU.mult, op1=ALU.add)

                W1aT_sb = [hw.tile([sz, 512], F32, tag=f"w1a{kb}", name=f"w1a{kb}_sb")
                           for kb, (_, off, sz) in enumerate(xcb)]
                for kb, (_, off, sz) in enumerate(xcb):
                    nc.sync.dma_start(W1aT_sb[kb][:], W1aT[off:off + sz, :])
                h1T = [hact.tile([128, QN], F32, tag=f"h1_{j}", name=f"h1_{j}_sb") for j in range(4)]
                for j in range(4):
                    jsl = slice(j * 128, (j + 1) * 128)
                    for cc in range(QN // 512):
                        csl = slice(cc * 512, (cc + 1) * 512)
                        p1 = hps.tile([128, 512], F32, tag="mm")
                        for kb, (pl_, off, sz) in enumerate(xcb):
                            nc.tensor.matmul(p1[:], W1aT_sb[kb][:, jsl], pl_[:, csl],
                                             start=(kb == 0), stop=(kb == len(xcb) - 1))
                        tt_ = htmp.tile([128, 512], F32, tag="tt")
                        nc.scalar.activation(tt_[:], p1[:], AF.Identity,
                                             scale=scl["s1r"][:, j:j + 1],
                                             bias=biasp[:, j:j + 1])
                        uu = htmp.tile([128, 512], F32, tag="uu")
                        nc.scalar.activation(uu[:], tt_[:], AF.Copy, scale=0.2)
                        nc.vector.tensor_tensor(h1T[j][:, csl], tt_[:], uu[:], op=ALU.max)

                W2T_sb = [hw.tile([128, 256], F32, tag=f"w2{kb}", name=f"w2{kb}_sb") for kb in range(4)]
                for kb in range(4):
                    nc.sync.dma_start(W2T_sb[kb][:], W2T[kb * 128:(kb + 1) * 128, :])
                h2T = [hact.tile([128, QN], F32, tag=f"h2_{j}", name=f"h2_{j}_sb") for j in range(2)]
                for j in range(2):
                    jsl = slice(j * 128, (j + 1) * 128)
                    for cc in range(QN // 512):
                        csl = slice(cc * 512, (cc + 1) * 512)
                        p2 = hps.tile([128, 512], F32, tag="mm")
                        for kb in range(4):
                            nc.tensor.matmul(p2[:], W2T_sb[kb][:, jsl], h1T[kb][:, csl],
                                             start=(kb == 0), stop=(kb == 3))
                        tt_ = htmp.tile([128, 512], F32, tag="tt")
                        nc.scalar.activation(tt_[:], p2[:], AF.Identity,
                                             scale=scl["s2r"][:, j:j + 1],
                                             bias=scl["b2r"][:, j:j + 1])
                        uu = htmp.tile([128, 512], F32, tag="uu")
                        nc.scalar.activation(uu[:], tt_[:], AF.Copy, scale=0.2)
                        nc.vector.tensor_tensor(h2T[j][:, csl], tt_[:], uu[:], op=ALU.max)

                W3T_sb = [hw.tile([128, 16], F32, tag=f"w3{kb}", name=f"w3{kb}_sb") for kb in range(2)]
                for kb in range(2):
                    nc.sync.dma_start(W3T_sb[kb][:], W3T[kb * 128:(kb + 1) * 128, :])
                oT = hact.tile([16, QN], F32)
                for cc in range(QN // 512):
                    csl = slice(cc * 512, (cc + 1) * 512)
                    p3 = hps.tile([16, 512], F32, tag="mm")
                    for kb in range(2):
                        nc.tensor.matmul(p3[:], W3T_sb[kb][:], h2T[kb][:, csl],
                                         start=(kb == 0), stop=(kb == 1))
                    nc.scalar.activation(oT[:, csl], p3[:], AF.Identity,
                                         bias=b3_sb[:, 0:1])
                for j in range(QT):
                    jsl = slice(j * 128, (j + 1) * 128)
                    po = hps1.tile([P, 16], F32, tag="tr")
                    nc.tensor.transpose(po[:], oT[:, jsl], ident[0:16, 0:16])
                    orow = htmp.tile([P, 16], F32, tag="orow")
                    nc.scalar.activation(orow[:], po[:], AF.Copy)
                    nc.sync.dma_start(outd[jsl, :], orow[:])

    nc.compile()
    return nc


def host_inputs(inputs):
    """Build the 8 per-core input maps from the full problem inputs."""
    x = np.asarray(inputs["x"], dtype=np.float32)
    common = {}
    for li, (C, O) in enumerate(LAYERS):
        W = np.asarray(inputs[f"W{li + 1}"], np.float32)
        s = np.asarray(inputs[f"s{li + 1}"], np.float32)
        b = np.asarray(inputs[f"b{li + 1}"], np.float32)
        Cc = W.shape[1] // 2
        assert Cc == C
        An = (s[:, None] * W[:, :C]).astype(np.float32)
        Qn = (s[:, None] * (W[:, C:] - W[:, :C])).astype(np.float32)
        common[f"AnT{li}"] = np.ascontiguousarray(An.T)
        common[f"QnT{li}"] = np.ascontiguousarray(Qn.T)
        common[f"brep{li}"] = np.broadcast_to(b, (P, O)).copy()
    common["WgT"] = np.ascontiguousarray(np.asarray(inputs["Wg"], np.float32).T)
    Ws1 = np.asarray(inputs["Ws1"], np.float32)
    common["W1aT"] = np.ascontiguousarray(Ws1[:, :512].T)
    common["W1bT"] = np.ascontiguousarray(Ws1[:, 512:].T)
    common["W2T"] = np.ascontiguousarray(np.asarray(inputs["Ws2"], np.float32).T)
    W3 = np.asarray(inputs["Ws3"], np.float32)          # [13, 256]
    W3p = np.zeros((16, 256), np.float32)
    W3p[:13] = W3
    common["W3T"] = np.ascontiguousarray(W3p.T)
    common["sgr"] = np.ascontiguousarray(np.asarray(inputs["sg"], np.float32).reshape(8, P).T)
    common["bgr"] = np.ascontiguousarray(np.asarray(inputs["bg"], np.float32).reshape(8, P).T)
    common["s1r"] = np.ascontiguousarray(np.asarray(inputs["ss1"], np.float32).reshape(4, P).T)
    common["b1r"] = np.ascontiguousarray(np.asarray(inputs["bs1"], np.float32).reshape(4, P).T)
    common["s2r"] = np.ascontiguousarray(np.asarray(inputs["ss2"], np.float32).reshape(2, P).T)
    common["b2r"] = np.ascontiguousarray(np.asarray(inputs["bs2"], np.float32).reshape(2, P).T)
    b3 = np.zeros((16, 1), np.float32)
    b3[:13, 0] = np.asarray(inputs["bs3"], np.float32)
    common["b3r"] = b3

    maps = []
    for c in range(8):
        b, q = c // NCHUNK, c % NCHUNK
        m = dict(common)
        m["xfT"] = np.ascontiguousarray(x[b].T)                      # [3, N]
        m["xqT"] = np.ascontiguousarray(x[b, q * QN:(q + 1) * QN].T)  # [3, QN]
        maps.append(m)
    return maps


_CACHED_NC = None


def kernel(**inputs) -> np.ndarray:
    global _CACHED_NC
    from concourse.bass_utils import run_bass_kernel_spmd

    if _CACHED_NC is None:
        _CACHED_NC = build_program()
    maps = host_inputs(inputs)
    res = run_bass_kernel_spmd(_CACHED_NC, maps, list(range(8)))
    out = np.zeros((B, N, 13), np.float32)
    for c in range(8):
        b, q = c // NCHUNK, c % NCHUNK
        out[b, q * QN:(q + 1) * QN, :] = res.results[c]["out"][:, :13]
    return out


if __name__ == "__main__":
    import reference

    inputs = {k: np.asarray(v) for k, v in reference.setup_inputs().items()}
    got = kernel(**inputs)
    print("kernel output", got.shape, got.dtype)


# revision 20
# speedup vs baseline: 1.1260x; 1.1260x over previous
"""DGCNN segmentation kernel for 8 Trainium2 NeuronCores.

Sharding: core c owns batch b = c // 4 and query chunk q = c % 4 (1024
queries).  Each layer: distances + exact top-20 selection (DVE max8 rounds)
+ indirect-DMA gather of projected neighbor features + max-over-k reduce.
Features are exchanged between the 4 cores of a batch with an AllGather
after layers 1-3; the global-max-pool vector is combined with an
AllReduce(max).

EdgeConv restructuring (host folds s into weights):
  y[n,k,:] = s * (W @ [nbr - xc; xc]) + b = An @ nbr + Qn @ xc + b
  with An = s*W[:, :C],  Qn = s*(W[:, C:] - W[:, :C]).
  max_k lrelu(y) = lrelu(max_k (An @ nbr) + Qn @ xc + b)   (monotonic).
"""
import sys

sys.path.insert(0, "/opt/trn_rl_repo")

import numpy as np

import concourse.bacc as bacc
import concourse.bass as bass
import concourse.mybir as mybir
import concourse.tile as tile
from concourse import masks

F32 = mybir.dt.float32
U32 = mybir.dt.uint32
AF = mybir.ActivationFunctionType
ALU = mybir.AluOpType

SEL_MODE = "sliced"   # "full" = 3x(max+mi+mr) full-width; "sliced" = 32x128 hierarchy
MARK = -3.0e38

N, K, B = 4096, 20, 2
NCHUNK = 4          # query chunks per batch
QN = N // NCHUNK    # queries per core
P = 128             # queries per tile
QT = QN // P        # query tiles per core
LAYERS = [(3, 64), (64, 64), (64, 128), (128, 256)]  # (C, O)
EMB = 1024
GROUPS = [[0, 1, 2, 3], [4, 5, 6, 7]]


def build_program():
    nc = bacc.Bacc("TRN2", target_bir_lowering=False, debug=False, num_devices=8)

    ins = {}

    def inp(name, shape):
        ins[name] = nc.dram_tensor(name, list(shape), F32, kind="ExternalInput")
        return ins[name]

    xfT = inp("xfT", [3, N])
    xqT = inp("xqT", [3, QN])
    for li, (C, O) in enumerate(LAYERS):
        inp(f"AnT{li}", [C, O])
        inp(f"QnT{li}", [C, O])
        inp(f"brep{li}", [P, O])
    WgT = inp("WgT", [512, EMB])
    W1aT = inp("W1aT", [512, 512])
    W1bT = inp("W1bT", [EMB, 512])
    W2T = inp("W2T", [512, 256])
    W3T = inp("W3T", [256, 16])      # 13 padded to 16 cols
    sgr = inp("sgr", [P, 8])
    bgr = inp("bgr", [P, 8])
    s1r = inp("s1r", [P, 4])
    b1r = inp("b1r", [P, 4])
    s2r = inp("s2r", [P, 2])
    b2r = inp("b2r", [P, 2])
    b3r = inp("b3r", [16, 1])

    outd = nc.dram_tensor("out", [QN, 16], F32, kind="ExternalOutput")
    dbg_s = nc.dram_tensor("dbg_s", [P, 512], F32, kind="ExternalOutput")
    dbg_nidx = nc.dram_tensor("dbg_nidx", [P, 20], U32, kind="ExternalOutput")
    dbg_zg = nc.dram_tensor("dbg_zg", [P, K * 64], F32, kind="ExternalOutput")
    dbg_x1 = nc.dram_tensor("dbg_x1", [64, QN], F32, kind="ExternalOutput")
    dbg_xt2 = nc.dram_tensor("dbg_xt2", [64, N], F32, kind="ExternalOutput")

    zdram = [nc.dram_tensor(f"zdram{li}", [N, O], F32) for li, (C, O) in enumerate(LAYERS)]
    ccin = [nc.dram_tensor(f"ccin{li}", [O, QN], F32)
            for li, (C, O) in enumerate(LAYERS[:3])]
    ccout = [nc.dram_tensor(f"ccout{li}", [NCHUNK, O, QN], F32)
             for li, (C, O) in enumerate(LAYERS[:3])]
    gin = nc.dram_tensor("gin", [8, P], F32)
    gout = nc.dram_tensor("gout", [8, P], F32)

    with tile.TileContext(nc) as tc:
        with (
            tc.tile_pool(name="perst", bufs=1) as perst,
            tc.tile_pool(name="wpool", bufs=1) as wpool,
        ):
            ident = perst.tile([128, 128], F32)
            masks.make_identity(nc, ident[:])
            onesP = perst.tile([128, 1], F32)
            nc.vector.memset(onesP[:], 1.0)
            ones1 = perst.tile([1, P], F32)
            nc.vector.memset(ones1[:], 1.0)

            # feature planes (transposed, this core's queries)
            x1T = perst.tile([64, QN], F32)
            x2T = perst.tile([64, QN], F32)
            plane2 = perst.tile([128, QN], F32)    # x3T
            plane3a = perst.tile([128, QN], F32)   # x4T[0:128]
            plane3b = perst.tile([128, QN], F32)   # x4T[128:256]
            xq_sb = perst.tile([3, QN], F32)
            nc.sync.dma_start(xq_sb[:], xqT[:])

            xT = perst.tile([128, N], F32)         # keys (C rows used)
            nxx = perst.tile([1, N], F32)          # -|x|^2 row

            if SEL_MODE == "sliced":
                sbase_i = perst.tile([128, 256], mybir.dt.int32)
                nc.gpsimd.iota(sbase_i[:], pattern=[[128, 32], [0, 8]],
                               base=0, channel_multiplier=0)
                sbase = perst.tile([128, 256], F32)
                nc.vector.tensor_copy(sbase[:], sbase_i[:])
                negbig = perst.tile([128, 256], F32)
                nc.vector.memset(negbig[:], MARK)

            my_src = {0: xq_sb[0:3, :], 1: x1T[:, :],
                      2: x2T[:, :], 3: plane2[0:128, :]}
            out_planes = {0: [x1T[:, :]], 1: [x2T[:, :]],
                          2: [plane2[0:128, :]], 3: [plane3a[0:128, :], plane3b[0:128, :]]}

            AnT_sb, QnT_sb, brep_sb = {}, {}, {}
            for li, (C, O) in enumerate(LAYERS):
                AnT_sb[li] = wpool.tile([C, O], F32, tag=f"AnT{li}", name=f"AnT{li}_sb")
                nc.sync.dma_start(AnT_sb[li][:], ins[f"AnT{li}"][:])
                QnT_sb[li] = wpool.tile([C, O], F32, tag=f"QnT{li}", name=f"QnT{li}_sb")
                nc.sync.dma_start(QnT_sb[li][:], ins[f"QnT{li}"][:])
                brep_sb[li] = wpool.tile([P, O], F32, tag=f"brep{li}", name=f"brep{li}_sb")
                nc.sync.dma_start(brep_sb[li][:], ins[f"brep{li}"][:])

            # ---------------- layers ----------------
            with (
                tc.tile_pool(name="lay", bufs=1) as lay,
                tc.tile_pool(name="Spool", bufs=2) as Spool,
                tc.tile_pool(name="zgpool", bufs=3) as zgpool,
                tc.tile_pool(name="small", bufs=2) as small,
                tc.tile_pool(name="ps1", bufs=1, space="PSUM") as ps1,
                tc.tile_pool(name="ps2", bufs=2, space="PSUM") as ps2,
            ):
                for li, (C, O) in enumerate(LAYERS):
                    # --- keys into xT ---
                    if li == 0:
                        nc.sync.dma_start(xT[0:3, :], xfT[:])
                    else:
                        cco = ccout[li - 1]
                        nc.sync.dma_start(
                            xT[0:C, :].rearrange("c (r j) -> c r j", r=NCHUNK),
                            cco[:, :, :].rearrange("r c j -> c r j"),
                        )
                    # --- -xx row ---
                    sq = lay.tile([128, N], F32, tag="sq")
                    nc.scalar.activation(sq[0:C, :], xT[0:C, :], AF.Square)
                    for ch in range(8):
                        sl = slice(ch * 512, (ch + 1) * 512)
                        pxx = ps1.tile([1, 512], F32, tag="pxx")
                        nc.tensor.matmul(pxx[:], onesP[0:C, :], sq[0:C, sl])
                        nc.scalar.activation(nxx[:, sl], pxx[:], AF.Copy, scale=-1.0)
                    # --- Z rows -> DRAM ---
                    for j in range(N // 128):
                        jsl = slice(j * 128, (j + 1) * 128)
                        pz = ps2.tile([128, O], F32, tag="pz")
                        nc.tensor.matmul(pz[:], xT[0:C, jsl], AnT_sb[li][:])
                        zrow = small.tile([128, O], F32, tag="zrow")
                        nc.scalar.activation(zrow[:], pz[:], AF.Copy)
                        nc.sync.dma_start(zdram[li][jsl, :], zrow[:])
                    # --- query tiles ---
                    for t in range(QT):
                        tsl = slice(t * P, (t + 1) * P)
                        src = my_src[li]
                        lhs2 = small.tile([C, P], F32, tag="lhs2")
                        nc.scalar.activation(lhs2[:], src[:, tsl], AF.Copy, scale=2.0)

                        S = Spool.tile([P, N], F32, tag="S")
                        for ch in range(8):
                            sl = slice(ch * 512, (ch + 1) * 512)
                            pd = ps2.tile([P, 512], F32, tag="pd")
                            nc.tensor.matmul(pd[:], lhs2[:], xT[0:C, sl],
                                             start=True, stop=False)
                            nc.tensor.matmul(pd[:], ones1[:], nxx[:, sl],
                                             start=False, stop=True)
                            nc.scalar.activation(S[:, sl], pd[:], AF.Copy)

                        NIDX = small.tile([P, 24], U32, tag="NIDX")
                        if SEL_MODE == "full":
                            V = small.tile([P, 8], F32, tag="V")
                            scr = small.tile([P, 8], U32, tag="scr")
                            for r in range(3):
                                nc.vector.max(V[:], S[:])
                                if r < 2:
                                    nc.vector.max_index(NIDX[:, r * 8:(r + 1) * 8], V[:], S[:])
                                    nc.vector.match_replace(S[:], V[:], S[:], MARK)
                                else:
                                    nc.vector.max_index(scr[:], V[:], S[:])
                            nc.vector.tensor_copy(NIDX[:, 16:20], scr[:, 0:4])
                        else:
                            # per-slice top-8 (exact: P(>8 of top-20 in one
                            # 128-slice) ~ 1.5e-7/row)
                            V16 = small.tile([P, 256], F32, tag="V16")
                            I16 = small.tile([P, 256], U32, tag="I16")
                            for s in range(32):
                                ssl = slice(s * 128, (s + 1) * 128)
                                osl = slice(s * 8, (s + 1) * 8)
                                nc.vector.max(V16[:, osl], S[:, ssl])
                            for s in range(32):
                                ssl = slice(s * 128, (s + 1) * 128)
                                osl = slice(s * 8, (s + 1) * 8)
                                nc.vector.max_index(I16[:, osl], V16[:, osl], S[:, ssl])
                            GIDX = small.tile([P, 256], F32, tag="GIDX")
                            nc.vector.tensor_copy(GIDX[:], I16[:])
                            nc.vector.tensor_tensor(GIDX[:], GIDX[:], sbase[:], op=ALU.add)
                            # merge: mark the top-20 values in V16
                            Vm = small.tile([P, 8], F32, tag="Vm")
                            nc.vector.max(Vm[:], V16[:])
                            nc.vector.match_replace(V16[:], Vm[:], V16[:], MARK)
                            nc.vector.max(Vm[:], V16[:])
                            nc.vector.match_replace(V16[:], Vm[:], V16[:], MARK)
                            nc.vector.max(Vm[:], V16[:])
                            rep8 = small.tile([P, 8], F32, tag="rep8")
                            nc.vector.memset(rep8[:], 3.0e38)
                            nc.vector.tensor_copy(rep8[:, 0:4], Vm[:, 0:4])
                            nc.vector.match_replace(V16[:], rep8[:], V16[:], MARK)
                            # indices of marked entries
                            maskt = small.tile([P, 256], mybir.dt.uint8, tag="maskt")
                            nc.vector.tensor_scalar(maskt[:], V16[:], MARK, None,
                                                    op0=ALU.is_equal)
                            IDXm = small.tile([P, 256], F32, tag="IDXm")
                            nc.vector.tensor_copy(IDXm[:], negbig[:])
                            nc.vector.copy_predicated(IDXm[:], maskt[:], GIDX[:])
                            NIDXf = small.tile([P, 24], F32, tag="NIDXf")
                            nc.vector.max(NIDXf[:, 0:8], IDXm[:])
                            nc.vector.match_replace(IDXm[:], NIDXf[:, 0:8], IDXm[:], MARK)
                            nc.vector.max(NIDXf[:, 8:16], IDXm[:])
                            nc.vector.match_replace(IDXm[:], NIDXf[:, 8:16], IDXm[:], MARK)
                            nc.vector.max(NIDXf[:, 16:24], IDXm[:])
                            nc.vector.tensor_copy(NIDX[:, 0:20], NIDXf[:, 0:20])

                        if li == 0 and t == 0:
                            nc.sync.dma_start(dbg_s[:, :], S[:, 0:512])
                            nc.sync.dma_start(dbg_nidx[:, :], NIDX[:, 0:20])
                        zg = zgpool.tile([P, K * O], F32, tag="zg")
                        # One offset per partition per instruction: the only
                        # layout the HW dynamic-AP path handles correctly.
                        for kk in range(K):
                            nc.gpsimd.indirect_dma_start(
                                out=zg[:, kk * O:(kk + 1) * O],
                                out_offset=None,
                                in_=zdram[li][:, :],
                                in_offset=bass.IndirectOffsetOnAxis(
                                    ap=NIDX[:, kk:kk + 1], axis=0),
                            )

                        if li == 0 and t == 0:
                            nc.sync.dma_start(dbg_zg[:, :], zg[:, :])
                        pq = ps1.tile([P, O], F32, tag="pq")
                        nc.tensor.matmul(pq[:], src[:, tsl], QnT_sb[li][:])
                        M = small.tile([P, O], F32, tag="M")
                        nc.vector.tensor_reduce(
                            M[:], zg[:].rearrange("p (k o) -> p o k", k=K),
                            axis=mybir.AxisListType.X, op=ALU.max)
                        y = small.tile([P, O], F32, tag="y")
                        nc.vector.tensor_tensor(y[:], M[:], pq[:], op=ALU.add)
                        nc.vector.tensor_tensor(y[:], y[:], brep_sb[li][:], op=ALU.add)
                        u = small.tile([P, O], F32, tag="u")
                        nc.scalar.activation(u[:], y[:], AF.Copy, scale=0.2)
                        xo = small.tile([P, O], F32, tag="xo")
                        nc.vector.tensor_tensor(xo[:], y[:], u[:], op=ALU.max)

                        for pl, dest in enumerate(out_planes[li]):
                            pt = ps1.tile([min(O, 128), P], F32, tag="pt")
                            nc.tensor.transpose(
                                pt[:], xo[:, pl * 128:pl * 128 + min(O, 128)], ident[:])
                            nc.scalar.activation(dest[:, tsl], pt[:], AF.Copy)

                    if li == 0:
                        nc.sync.dma_start(dbg_x1[:, :], x1T[:, :])
                    if li == 1:
                        nc.sync.dma_start(dbg_xt2[:, :], xT[0:64, :])
                    if li < 3:
                        nc.sync.dma_start(ccin[li][:, :], out_planes[li][0])
                        nc.gpsimd.collective_compute(
                            "AllGather", ALU.bypass, replica_groups=GROUPS,
                            ins=[ccin[li][:, :]], outs=[ccout[li][:, :, :]])

            # ---------------- head ----------------
            # K-blocks of xcat: (plane AP, weight-row offset, size)
            xcb = [(x1T[:, :], 0, 64), (x2T[:, :], 64, 64), (plane2[:, :], 128, 128),
                   (plane3a[:, :], 256, 128), (plane3b[:, :], 384, 128)]
            with (
                tc.tile_pool(name="hw", bufs=1) as hw,
                tc.tile_pool(name="hact", bufs=1) as hact,
                tc.tile_pool(name="htmp", bufs=2) as htmp,
                tc.tile_pool(name="hps", bufs=2, space="PSUM") as hps,
                tc.tile_pool(name="hps1", bufs=1, space="PSUM") as hps1,
            ):
                WgT_sb = [hw.tile([sz, EMB], F32, tag=f"wg{kb}", name=f"wg{kb}_sb")
                          for kb, (_, off, sz) in enumerate(xcb)]
                for kb, (_, off, sz) in enumerate(xcb):
                    nc.sync.dma_start(WgT_sb[kb][:], WgT[off:off + sz, :])
                scl = {}
                for nm, t_ in (("sgr", sgr), ("bgr", bgr), ("s1r", s1r), ("b1r", b1r),
                               ("s2r", s2r), ("b2r", b2r)):
                    scl[nm] = hw.tile(list(t_.shape), F32, tag=nm, name=nm + "_sb")
                    nc.sync.dma_start(scl[nm][:], t_[:])
                b3_sb = hw.tile([16, 1], F32, tag="b3")
                nc.sync.dma_start(b3_sb[:], b3r[:])

                gmax = hact.tile([P, 8], F32)
                for i in range(8):
                    isl = slice(i * 128, (i + 1) * 128)
                    gT = htmp.tile([128, QN], F32, tag="gT")
                    for cc in range(QN // 512):
                        csl = slice(cc * 512, (cc + 1) * 512)
                        pg = hps.tile([128, 512], F32, tag="mm")
                        for kb, (pl_, off, sz) in enumerate(xcb):
                            nc.tensor.matmul(pg[:], WgT_sb[kb][:, isl], pl_[:, csl],
                                             start=(kb == 0), stop=(kb == len(xcb) - 1))
                        # t = sg*mm + bg ; lrelu
                        tt_ = htmp.tile([128, 512], F32, tag="tt")
                        nc.scalar.activation(tt_[:], pg[:], AF.Identity,
                                             scale=scl["sgr"][:, i:i + 1],
                                             bias=scl["bgr"][:, i:i + 1])
                        uu = htmp.tile([128, 512], F32, tag="uu")
                        nc.scalar.activation(uu[:], tt_[:], AF.Copy, scale=0.2)
                        nc.vector.tensor_tensor(gT[:, csl], tt_[:], uu[:], op=ALU.max)
                    nc.vector.tensor_reduce(gmax[:, i:i + 1], gT[:],
                                            axis=mybir.AxisListType.X, op=ALU.max)
                # AllReduce max over the 4 cores of the batch
                pgt = hps1.tile([8, P], F32, tag="tr")
                nc.tensor.transpose(pgt[:], gmax[:], ident[:])
                gt8 = hact.tile([8, P], F32)
                nc.scalar.activation(gt8[:], pgt[:], AF.Copy)
                nc.sync.dma_start(gin[:, :], gt8[:])
                nc.gpsimd.collective_compute(
                    "AllReduce", ALU.max, replica_groups=GROUPS,
                    ins=[gin[:, :]], outs=[gout[:, :]])
                gld = hact.tile([8, P], F32)
                nc.sync.dma_start(gld[:], gout[:, :])
                pgb = hps1.tile([P, 8], F32, tag="tr")
                nc.tensor.transpose(pgb[:], gld[:], ident[0:8, 0:8])
                gsb = hact.tile([P, 8], F32)
                nc.scalar.activation(gsb[:], pgb[:], AF.Copy)

                W1bT_sb = [hw.tile([128, 512], F32, tag=f"w1b{i}", name=f"w1b{i}_sb") for i in range(8)]
                for i in range(8):
                    nc.sync.dma_start(W1bT_sb[i][:], W1bT[i * 128:(i + 1) * 128, :])
                # bias' = s1*(W1b@g) + b1 per out-plane j
                biasp = hact.tile([P, 4], F32)
                for j in range(4):
                    jsl = slice(j * 128, (j + 1) * 128)
                    pb = hps1.tile([P, 1], F32, tag="pb")
                    for i in range(8):
                        nc.tensor.matmul(pb[:], W1bT_sb[i][:, jsl], gsb[:, i:i + 1],
                                         start=(i == 0), stop=(i == 7))
                    nc.vector.scalar_tensor_tensor(
                        out=biasp[:, j:j + 1], in0=pb[:], scalar=scl["s1r"][:, j:j + 1],
                        in1=scl["b1r"][:, j:j + 1], op0=ALU.mult, op1=ALU.add)

                W1aT_sb = [hw.tile([sz, 512], F32, tag=f"w1a{kb}", name=f"w1a{kb}_sb")
                           for kb, (_, off, sz) in enumerate(xcb)]
                for kb, (_, off, sz) in enumerate(xcb):
                    nc.sync.dma_start(W1aT_sb[kb][:], W1aT[off:off + sz, :])
                h1T = [hact.tile([128, QN], F32, tag=f"h1_{j}", name=f"h1_{j}_sb") for j in range(4)]
                for j in range(4):
                    jsl = slice(j * 128, (j + 1) * 128)
                    for cc in range(QN // 512):
                        csl = slice(cc * 512, (cc + 1) * 512)
                        p1 = hps.tile([128, 512], F32, tag="mm")
                        for kb, (pl_, off, sz) in enumerate(xcb):
                            nc.tensor.matmul(p1[:], W1aT_sb[kb][:, jsl], pl_[:, csl],
                                             start=(kb == 0), stop=(kb == len(xcb) - 1))
                        tt_ = htmp.tile([128, 512], F32, tag="tt")
                        nc.scalar.activation(tt_[:], p1[:], AF.Identity,
                                             scale=scl["s1r"][:, j:j + 1],
                                             bias=biasp[:, j:j + 1])
                        uu = htmp.tile([128, 512], F32, tag="uu")
                        nc.scalar.activation(uu[:], tt_[:], AF.Copy, scale=0.2)
                        nc.vector.tensor_tensor(h1T[j][:, csl], tt_[:], uu[:], op=ALU.max)

                W2T_sb = [hw.tile([128, 256], F32, tag=f"w2{kb}", name=f"w2{kb}_sb") for kb in range(4)]
                for kb in range(4):
                    nc.sync.dma_start(W2T_sb[kb][:], W2T[kb * 128:(kb + 1) * 128, :])
                h2T = [hact.tile([128, QN], F32, tag=f"h2_{j}", name=f"h2_{j}_sb") for j in range(2)]
                for j in range(2):
                    jsl = slice(j * 128, (j + 1) * 128)
                    for cc in range(QN // 512):
                        csl = slice(cc * 512, (cc + 1) * 512)
                        p2 = hps.tile([128, 512], F32, tag="mm")
                        for kb in range(4):
                            nc.tensor.matmul(p2[:], W2T_sb[kb][:, jsl], h1T[kb][:, csl],
                                             start=(kb == 0), stop=(kb == 3))
                        tt_ = htmp.tile([128, 512], F32, tag="tt")
                        nc.scalar.activation(tt_[:], p2[:], AF.Identity,
                                             scale=scl["s2r"][:, j:j + 1],
                                             bias=scl["b2r"][:, j:j + 1])
                        uu = htmp.tile([128, 512], F32, tag="uu")
                        nc.scalar.activation(uu[:], tt_[:], AF.Copy, scale=0.2)
                        nc.vector.tensor_tensor(h2T[j][:, csl], tt_[:], uu[:], op=ALU.max)

                W3T_sb = [hw.tile([128, 16], F32, tag=f"w3{kb}", name=f"w3{kb}_sb") for kb in range(2)]
                for kb in range(2):
                    nc.sync.dma_start(W3T_sb[kb][:], W3T[kb * 128:(kb + 1) * 128, :])
                oT = hact.tile([16, QN], F32)
                for cc in range(QN // 512):
                    csl = slice(cc * 512, (cc + 1) * 512)
                    p3 = hps.tile([16, 512], F32, tag="mm")
                    for kb in range(2):
                        nc.tensor.matmul(p3[:], W3T_sb[kb][:], h2T[kb][:, csl],
                                         start=(kb == 0), stop=(kb == 1))
                    nc.scalar.activation(oT[:, csl], p3[:], AF.Identity,
                                         bias=b3_sb[:, 0:1])
                for j in range(QT):
                    jsl = slice(j * 128, (j + 1) * 128)
                    po = hps1.tile([P, 16], F32, tag="tr")
                    nc.tensor.transpose(po[:], oT[:, jsl], ident[0:16, 0:16])
                    orow = htmp.tile([P, 16], F32, tag="orow")
                    nc.scalar.activation(orow[:], po[:], AF.Copy)
                    nc.sync.dma_start(outd[jsl, :], orow[:])

    nc.compile()
    return nc


def host_inputs(inputs):
    """Build the 8 per-core input maps from the full problem inputs."""
    x = np.asarray(inputs["x"], dtype=np.float32)
    common = {}
    for li, (C, O) in enumerate(LAYERS):
        W = np.asarray(inputs[f"W{li + 1}"], np.float32)
        s = np.asarray(inputs[f"s{li + 1}"], np.float32)
        b = np.asarray(inputs[f"b{li + 1}"], np.float32)
        Cc = W.shape[1] // 2
        assert Cc == C
        An = (s[:, None] * W[:, :C]).astype(np.float32)
        Qn = (s[:, None] * (W[:, C:] - W[:, :C])).astype(np.float32)
        common[f"AnT{li}"] = np.ascontiguousarray(An.T)
        common[f"QnT{li}"] = np.ascontiguousarray(Qn.T)
        common[f"brep{li}"] = np.broadcast_to(b, (P, O)).copy()
    common["WgT"] = np.ascontiguousarray(np.asarray(inputs["Wg"], np.float32).T)
    Ws1 = np.asarray(inputs["Ws1"], np.float32)
    common["W1aT"] = np.ascontiguousarray(Ws1[:, :512].T)
    common["W1bT"] = np.ascontiguousarray(Ws1[:, 512:].T)
    common["W2T"] = np.ascontiguousarray(np.asarray(inputs["Ws2"], np.float32).T)
    W3 = np.asarray(inputs["Ws3"], np.float32)          # [13, 256]
    W3p = np.zeros((16, 256), np.float32)
    W3p[:13] = W3
    common["W3T"] = np.ascontiguousarray(W3p.T)
    common["sgr"] = np.ascontiguousarray(np.asarray(inputs["sg"], np.float32).reshape(8, P).T)
    common["bgr"] = np.ascontiguousarray(np.asarray(inputs["bg"], np.float32).reshape(8, P).T)
    common["s1r"] = np.ascontiguousarray(np.asarray(inputs["ss1"], np.float32).reshape(4, P).T)
    common["b1r"] = np.ascontiguousarray(np.asarray(inputs["bs1"], np.float32).reshape(4, P).T)
    common["s2r"] = np.ascontiguousarray(np.asarray(inputs["ss2"], np.float32).reshape(2, P).T)
    common["b2r"] = np.ascontiguousarray(np.asarray(inputs["bs2"], np.float32).reshape(2, P).T)
    b3 = np.zeros((16, 1), np.float32)
    b3[:13, 0] = np.asarray(inputs["bs3"], np.float32)
    common["b3r"] = b3

    maps = []
    for c in range(8):
        b, q = c // NCHUNK, c % NCHUNK
        m = dict(common)
        m["xfT"] = np.ascontiguousarray(x[b].T)                      # [3, N]
        m["xqT"] = np.ascontiguousarray(x[b, q * QN:(q + 1) * QN].T)  # [3, QN]
        maps.append(m)
    return maps


_CACHED_NC = None


def kernel(**inputs) -> np.ndarray:
    global _CACHED_NC
    from concourse.bass_utils import run_bass_kernel_spmd

    if _CACHED_NC is None:
        _CACHED_NC = build_program()
    maps = host_inputs(inputs)
    res = run_bass_kernel_spmd(_CACHED_NC, maps, list(range(8)))
    out = np.zeros((B, N, 13), np.float32)
    for c in range(8):
        b, q = c // NCHUNK, c % NCHUNK
        out[b, q * QN:(q + 1) * QN, :] = res.results[c]["out"][:, :13]
    return out


if __name__ == "__main__":
    import reference

    inputs = {k: np.asarray(v) for k, v in reference.setup_inputs().items()}
    got = kernel(**inputs)
    print("kernel output", got.shape, got.dtype)
